# revision 1
# baseline (speedup 1.0000x reference)
"""DeformConv1d Trainium2 kernel (banded-matmul formulation).

Math (exact rewrite of the reference):
  s_k[t]   = clip(offsets[t,k] + k, 0, 2)
  A_kd[t]  = max(0, 1 - |s_k[t] - d|),  d in {0,1,2}   (tent weights; d=3 is 0)
  interp[(c,k), t] = sum_d A_kd[t] * x[c, t+d]
  out[o,t] = sum_{c,k} W[o,c,k] * interp[(c,k), t] + bias[o]

The interp stage runs on the TensorEngine as a banded matmul: per 126-wide
t-tile, B_k[t', t] = A_{k, t'-t}[t] is a 3-diagonal band matrix built from
per-partition tensor_scalar ops, and interp = xT_tile^T @ B_k.

Sharding: data-parallel over batch B=8 across 8 NeuronCores.
"""

import numpy as np

B, C, L = 8, 256, 4096
C_OUT = 256
T = 4094
K = 3
ND = 3
TW = 126        # banded tile width (stride); tiles overlap by 2 in t'
NT = 33         # ceil(4094 / 126)
MW = 504        # main-matmul N chunk
NC9 = 9         # ceil(4094 / 504)

_CACHE = {}


def _build_bass():
    import concourse.bass as bass
    import concourse.mybir as mybir
    from concourse.tile import TileContext
    from concourse.vector_clock import ScopedClock

    def _patched_drain(self, tick_clock, wait_clock):
        drain_inst = self.nc.sync.drain()
        wait_clock.add_sem_waits(
            drain_inst.ins, ScopedClock({None: tick_clock.global_clock})
        )
        si = drain_inst.ins.sync_info
        waits = list(si.on_wait) if (si and si.on_wait) else []
        if len(waits) > 1:
            drain_inst.ins.sync_info = mybir.SyncInfo(
                on_wait=[waits[0]], on_update=[]
            )
            for w in waits[1:]:
                nop = self.nc.sync.nop()
                nop.ins.sync_info = mybir.SyncInfo(on_wait=[w], on_update=[])
        self.nc.all_engine_barrier()
        popped = self.nc._tile_sem_poison_stack.pop()
        assert popped is self._sem_poison
        self.nc.clear_and_free_semaphores(list(self.sems.allocated().values()))
        self.nc.all_engine_barrier()

    TileContext._drain_and_barrier = _patched_drain

    def _split_excess_waits(nc, maxw=1):
        ctr = [0]
        for fn in nc.m.functions:
            for bb in fn.blocks:
                insts = list(bb.instructions)
                out, changed = [], False
                for inst in insts:
                    si = inst.sync_info
                    waits = list(si.on_wait) if (si and si.on_wait) else []
                    if len(waits) > maxw:
                        for w in waits[:-maxw]:
                            nop = mybir.InstNoOp(
                                name=f"I-wsplit{ctr[0]}", ins=[], outs=[]
                            )
                            ctr[0] += 1
                            nop.engine = inst.engine
                            nop.sync_info = mybir.SyncInfo(
                                on_wait=[w], on_update=[]
                            )
                            out.append(nop)
                        inst.sync_info = mybir.SyncInfo(
                            on_wait=waits[-maxw:],
                            on_update=list(si.on_update) if si.on_update else [],
                        )
                        changed = True
                    out.append(inst)
                if changed:
                    bb.instructions = out

    fp32 = mybir.dt.float32
    bf16 = mybir.dt.bfloat16
    Op = mybir.AluOpType

    nc = bass.Bass()
    xtp_d = nc.dram_tensor("xtp", [128, NT * 256], bf16, kind="ExternalInput")
    offt_d = nc.dram_tensor("offt", [128, 3 * NT * K], fp32, kind="ExternalInput")
    wt_d = nc.dram_tensor("wt", [128, 12 * 128], bf16, kind="ExternalInput")
    diag_d = nc.dram_tensor("diagc", [128, 3 * TW], bf16, kind="ExternalInput")
    bias_d = nc.dram_tensor("bias", [128, 2], fp32, kind="ExternalInput")
    out_d = nc.dram_tensor("out", [C_OUT, T], fp32, kind="ExternalOutput")

    with TileContext(nc) as tc:
        with (
            tc.tile_pool(name="persist", bufs=1) as persist,
            tc.tile_pool(name="ipool", bufs=1) as ipool,
            tc.tile_pool(name="small", bufs=1) as small,
            tc.tile_pool(name="bpool", bufs=4) as bpool,
            tc.tile_pool(name="work", bufs=2) as work,
            tc.tile_pool(name="psum_i", bufs=5, space="PSUM") as psum_i,
            tc.tile_pool(name="psum_o", bufs=3, space="PSUM") as psum_o,
        ):
            xsb = persist.tile([128, NT * 256], bf16, name="xsb", tag="xsb")
            nc.sync.dma_start(xsb[:, :], xtp_d[:, :])
            wsb = persist.tile([128, 12 * 128], bf16, name="wsb", tag="wsb")
            nc.sync.dma_start(wsb[:, :], wt_d[:, :])
            diag = small.tile([128, 3, TW], bf16, name="diag", tag="diag")
            nc.sync.dma_start(
                diag[:, :, :], diag_d[:, :].rearrange("p (d t) -> p d t", d=3)
            )
            offt = small.tile([128, 3, NT, K], fp32, name="offt", tag="offt")
            nc.sync.dma_start(
                offt[:, :, :, :],
                offt_d[:, :].rearrange("p (d j k) -> p d j k", d=3, k=K),
            )
            bsb = small.tile([128, 2], fp32, name="bsb", tag="bsb")
            nc.sync.dma_start(bsb[:, :], bias_d[:, :])

            # ---- tent weights A'_{k,d}[p, j] = A_kd[126 j + p - d] ----
            ap = {}
            for d in range(ND):
                for k in range(K):
                    s = small.tile([128, NT], fp32, name=f"s{d}{k}", tag=f"s{d}{k}")
                    nc.vector.tensor_scalar(
                        s[:, :], offt[:, d, :, k], 1.0, float(k), Op.mult, Op.add
                    )
                    nc.vector.tensor_scalar(s[:, :], s[:, :], 0.0, None, Op.max)
                    nc.vector.tensor_scalar(s[:, :], s[:, :], 2.0, None, Op.min)
                    u = small.tile([128, NT], fp32, name=f"u{d}{k}", tag=f"u{d}{k}")
                    a = small.tile([128, NT], fp32, name=f"ap{d}{k}", tag=f"ap{d}{k}")
                    nc.vector.tensor_scalar(
                        u[:, :], s[:, :], -1.0, 1.0 + d, Op.mult, Op.add
                    )
                    nc.vector.tensor_scalar(
                        a[:, :], s[:, :], 1.0, 1.0 - d, Op.mult, Op.add
                    )
                    nc.vector.tensor_tensor(a[:, :], a[:, :], u[:, :], Op.min)
                    nc.vector.tensor_scalar(a[:, :], a[:, :], 0.0, None, Op.max)
                    ap[(k, d)] = a

            # ---- interp: B band build (DVE + GPSIMD) + PE matmul ----
            interp = {
                (ch, k): ipool.tile(
                    [128, NT * TW], bf16, name=f"it{ch}{k}", tag=f"it{ch}{k}"
                )
                for ch in range(2)
                for k in range(K)
            }
            from concourse.bass import broadcast_tensor_aps

            for j in range(NT):
                bt = bpool.tile([128, 3 * TW], bf16, name="bt", tag="bt")
                for k in range(K):
                    sl = slice(TW * k, TW * (k + 1))
                    # d=0 band on GPSIMD (frees DVE); A' column broadcast
                    # along the free dim (step-0 AP).
                    d0, a0 = broadcast_tensor_aps(
                        diag[:, 0, :], ap[(k, 0)][:, j : j + 1]
                    )
                    nc.gpsimd.tensor_tensor(bt[:, sl], d0, a0, Op.mult)
                    for d in (1, 2):
                        nc.vector.scalar_tensor_tensor(
                            bt[:, sl],
                            diag[:, d, :],
                            ap[(k, d)][:, j : j + 1],
                            bt[:, sl],
                            Op.mult,
                            Op.add,
                        )
                for ch in range(2):
                    pi = psum_i.tile([128, 3 * TW], fp32, name="pi", tag="pi")
                    nc.tensor.matmul(
                        pi[:, :],
                        xsb[:, 256 * j + 128 * ch : 256 * j + 128 * (ch + 1)],
                        bt[:, :],
                        start=True,
                        stop=True,
                    )
                    for k in range(K):
                        nc.scalar.copy(
                            interp[(ch, k)][:, TW * j : TW * (j + 1)],
                            pi[:, TW * k : TW * (k + 1)],
                        )

            # ---- main matmul: out = Wflat @ interp + bias ----
            # chunk-groups of 3: same W block feeds 3 consecutive matmuls so
            # the PE reorder window overlaps LDWEIGHTS, and PE bursts stay
            # dense for HAM.
            for oh in range(2):
                for cg in range(3):
                    chunks = [3 * cg + i for i in range(3)]
                    pos, ws = [], []
                    for c9 in chunks:
                        n0 = MW * c9
                        ws.append(min(MW, T - n0))
                        pos.append(
                            psum_o.tile([128, MW], fp32, name="po", tag="po")
                        )
                    for bi, (k, ch) in enumerate(
                        [(k, ch) for k in range(K) for ch in range(2)]
                    ):
                        idx = (k * 2 + ch) * 2 + oh
                        for ci, c9 in enumerate(chunks):
                            n0 = MW * c9
                            w = ws[ci]
                            nc.tensor.matmul(
                                pos[ci][:, :w],
                                wsb[:, 128 * idx : 128 * (idx + 1)],
                                interp[(ch, k)][:, n0 : n0 + w],
                                start=(bi == 0),
                                stop=(bi == 5),
                            )
                    for ci, c9 in enumerate(chunks):
                        n0 = MW * c9
                        w = ws[ci]
                        ost = work.tile([128, MW], fp32, name="ost", tag="ost")
                        nc.scalar.activation(
                            ost[:, :w], pos[ci][:, :w],
                            mybir.ActivationFunctionType.Identity,
                            bias=bsb[:, oh : oh + 1], scale=1.0,
                        )
                        nc.sync.dma_start(
                            out_d[128 * oh : 128 * (oh + 1), n0 : n0 + w],
                            ost[:, :w],
                        )
    _split_excess_waits(nc)
    return nc


def _prep_inputs(x, offsets, weight, bias):
    import ml_dtypes

    bf = ml_dtypes.bfloat16
    # xtp[p, j, c] = x[c, 126 j + p]  (zero beyond L)
    jj, pp = np.meshgrid(np.arange(NT), np.arange(128), indexing="ij")
    tt = TW * jj + pp  # [NT, 128]
    valid = tt < L
    tt_c = np.clip(tt, 0, L - 1)
    xtp_all = np.zeros((B, 128, NT, 256), np.float32)
    for b in range(B):
        g = x[b][:, tt_c]  # [256, NT, 128]
        g = np.where(valid[None, :, :], g, 0.0)
        xtp_all[b] = g.transpose(2, 1, 0)  # [128, NT, 256]
    xtp = xtp_all.reshape(B, 128, NT * 256).astype(bf)

    # offt[p, d, j, k] = offsets[126 j + p - d, k]  (zero out of range)
    offt_all = np.zeros((B, 128, 3, NT, K), np.float32)
    for d in range(3):
        t2 = TW * jj + pp - d
        v2 = (t2 >= 0) & (t2 < T)
        t2c = np.clip(t2, 0, T - 1)
        for b in range(B):
            g = offsets[b, 0][t2c, :]  # [NT, 128, K]
            g = np.where(v2[:, :, None], g, 0.0)
            offt_all[b, :, d] = g.transpose(1, 0, 2)
    offt = offt_all.reshape(B, 128, 3 * NT * K).astype(np.float32)

    wtt = np.zeros((128, K, 2, 2, 128), np.float32)
    for k in range(K):
        for ch in range(2):
            for oh in range(2):
                wtt[:, k, ch, oh, :] = weight[
                    128 * oh : 128 * (oh + 1), 128 * ch : 128 * (ch + 1), k
                ].T
    wt = wtt.reshape(128, 12 * 128).astype(bf)

    diagc = np.zeros((128, 3, TW), np.float32)
    for d in range(3):
        for t in range(TW):
            if t + d < 128:
                diagc[t + d, d, t] = 1.0
    diagc = diagc.reshape(128, 3 * TW).astype(bf)

    bias2 = bias.reshape(2, 128).T.astype(np.float32).copy()

    maps = []
    for b in range(B):
        maps.append(
            {
                "xtp": np.ascontiguousarray(xtp[b]),
                "offt": np.ascontiguousarray(offt[b]),
                "wt": wt,
                "diagc": diagc,
                "bias": bias2,
            }
        )
    return maps


def kernel(x, offsets, weight, bias):
    from concourse import bass_utils

    x = np.asarray(x, np.float32)
    offsets = np.asarray(offsets, np.float32)
    weight = np.asarray(weight, np.float32)
    bias = np.asarray(bias, np.float32)

    if "nc" not in _CACHE:
        _CACHE["nc"] = _build_bass()
    nc = _CACHE["nc"]
    in_maps = _prep_inputs(x, offsets, weight, bias)
    res = bass_utils.run_bass_kernel_spmd(nc, in_maps, core_ids=list(range(B)))
    out = np.stack([res.results[b]["out"] for b in range(B)], axis=0)
    return out.astype(np.float32)

